# revision 1
# baseline (speedup 1.0000x reference)
"""CRF negative log-likelihood on 8 TRN2 NeuronCores.

Data-parallel over batch (128 rows/core); each core runs an identical
independent program (no collectives) and the loss only needs batch means, so
per-core partial sums are combined in numpy. Per core:

  Forward algorithm in exp space, as a 512-step PE<->DVE recurrence:
    beta' = (E^T beta) * exp(em_s - 4.5)
  with E = exp(transitions) held as a stationary bf16 matmul weight,
  augmented with a ones column at col 64 so PSUM row 64 of every product is
  the per-batch normalizer sum(beta) for free. The -4.5 exp bias keeps the
  per-step growth ~flat (host adds 512*4.5 back). The batch is split into
  NCH=2 independent chains so the PE->sem->DVE->sem->PE dependency cycle of
  one chain hides under the other's engine work (the cycle, ~585ns/step, is
  the kernel's wall-time floor; more chains raise the DVE PSUM-access cost
  faster than they hide latency).

  Rescaling (fp32 range control) every KRS steps: the normalizer row is
  captured, reciprocal'd (DVE), partition-broadcast (GpSimd), and applied
  OFF the critical path by multiplying a FUTURE step's exp(emissions) tile,
  so the recurrence never stalls; captured Z values get one batched Ln at
  the very end (exactly one extra ACT table load, keeping Exp/Copy resident
  in the activation LUT the whole run).

  Emissions are DMAed once into a [128, 64, 64]-padded resident layout;
  PE transposes fill a [128, 4, 128] PSUM tile (one full bank) and a single
  ACT exp then produces EIGHT timesteps of F at 64-aligned partitions,
  amortizing the ACT access overhead. The gold-score section is emitted
  AFTER the forward loop so the scheduler prioritizes filling the
  recurrence pipeline at startup; gold work back-fills engine gaps.

  Gold score without gathers:
   - one-hot(tags): GpSimd broadcasts int16 tags across the tag axis, DVE
     is_equal against an int16 iota runs in the 2x all-2-byte perf mode;
   - emission term: em*onehot on GpSimd, free-axis-accumulated by ACT Copy;
   - transition term: PSUM-accumulated [oh_s, oh_{s+1}] outer-product
     matmuls build a global 48x48 transition count matrix (17-step
     overlapping one-hot tiles cover the chunk-boundary pairs), read out as
     a trace against a block-diag copy of `transitions`.
"""

import numpy as np

B, S, NT = 1024, 512, 48
NCORES = 8
BL = B // NCORES  # 128 batch rows per core
CH = 16    # gold-score chunk (steps per one-hot tile)
KRS = 256   # rescale period
EMT = 64   # steps per resident emissions tile
NCH = 2    # independent forward chains (batch split)
CWS_OVERRIDE = [64,64]  # optional explicit chain widths
EXP_BIAS = 4.5  # subtracted inside exp; host adds S*EXP_BIAS back

_CACHE = {}
_LABELS = {}


def _L(instr, label):
    try:
        _LABELS[instr.ins.name] = label
    except Exception:
        pass
    return instr


def _build_nc():
    import concourse.mybir as mybir
    from concourse import bacc
    from concourse import tile

    f32 = mybir.dt.float32
    bf16 = mybir.dt.bfloat16
    i32 = mybir.dt.int32
    AF = mybir.ActivationFunctionType
    OP = mybir.AluOpType

    nc = bacc.Bacc("TRN2", target_bir_lowering=False, debug=False,
                   num_devices=NCORES)

    em_d = nc.dram_tensor("em", [BL, S, NT], f32, kind="ExternalInput")
    tg_d = nc.dram_tensor("tg", [BL, S], i32, kind="ExternalInput")
    cst_d = nc.dram_tensor("consts", [128, 418], f32, kind="ExternalInput")

    logz_d = nc.dram_tensor("logz", [1, BL], f32, kind="ExternalOutput")
    gem_d = nc.dram_tensor("goldem", [BL, 1], f32, kind="ExternalOutput")
    gtr_d = nc.dram_tensor("goldtr", [96, 1], f32, kind="ExternalOutput")

    NRS = S // KRS
    if CWS_OVERRIDE:
        cws = list(CWS_OVERRIDE)
    else:
        base = BL // NCH
        cws = [base + (1 if i < BL % NCH else 0) for i in range(NCH)]
    offs = [sum(cws[:i]) for i in range(NCH)]

    with tile.TileContext(nc) as tc:
        with (
            tc.tile_pool(name="const", bufs=1) as cpool,
            tc.tile_pool(name="emres", bufs=S // EMT) as empool,
            tc.tile_pool(name="oh", bufs=4) as ohpool,
            tc.tile_pool(name="fwd", bufs=3) as fpool,
            tc.tile_pool(name="beta", bufs=3) as bpool,
            tc.tile_pool(name="small", bufs=4) as spool,
            tc.tile_pool(name="junk", bufs=3) as jpool,
            tc.tile_pool(name="pst", bufs=2, space="PSUM") as psT,
            tc.tile_pool(name="psp", bufs=5, space="PSUM") as psP,
            tc.tile_pool(name="psc", bufs=1, space="PSUM") as psC,
        ):
            # ---- constants: one packed DMA ----
            cst = cpool.tile([128, 418], f32, tag="cst")
            nc.sync.dma_start(out=cst[:], in_=cst_d[:])
            ident = cst[:, 0:128]
            eaug_f = cst[0:NT, 128:193]
            t2 = cst[0:96, 193:289]
            eaug = cpool.tile([NT, 65], bf16, tag="eaug")
            nc.scalar.activation(eaug[:], eaug_f, AF.Copy)
            i16 = mybir.dt.int16
            iota = cpool.tile([BL, CH + 1, NT], i16, tag="iota")
            nc.gpsimd.iota(iota[:], pattern=[[0, CH + 1], [1, NT]], base=0,
                           channel_multiplier=0)
            tg = cpool.tile([BL, S], i32, tag="tg")
            nc.sync.dma_start(out=tg[:], in_=tg_d[:])
            tg16 = cpool.tile([BL, S], i16, tag="tg16")
            nc.vector.tensor_copy(tg16[:], tg[:])
            bias_ap = cpool.tile([128, 1], f32, tag="bias")
            nc.gpsimd.memset(bias_ap[:], -EXP_BIAS)

            # ---- resident emissions, padded to 64 per step ----
            emp = []
            for t in range(S // EMT):
                et = empool.tile([BL, EMT, 64], f32, tag="em")
                nc.sync.dma_start(out=et[:, :, 0:NT],
                                  in_=em_d[:, t * EMT:(t + 1) * EMT, :])
                emp.append(et)

            # ---- forward state init ----
            betas = []
            for ch in range(NCH):
                b0 = bpool.tile([NT, cws[ch]], bf16, tag=f"beta{ch}")
                nc.vector.memset(b0[:], 0.0)
                nc.vector.memset(b0[0:1, :], 1.0)
                betas.append(b0)
            # Z capture buffer: NRS rescale slots + 1 final, on partition 64
            zbuf = cpool.tile([65, (NRS + 1) * BL], f32, tag="zbuf")

            # ---- forward loop: one exp per DOUBLE step-pair (4 steps) ----
            pending = {}  # pair index -> list of (chain, psb_tile)

            def make_f4(q):
                pst = psT.tile([128, 4, BL], f32, tag="pst")
                for u in (0, 1, 2, 3):
                    p = 4 * q + u
                    te, po = divmod(p, EMT // 2)
                    _L(nc.tensor.transpose(pst[:, u, 0:64],
                                           emp[te][:, 2 * po:2 * po + 2, :],
                                        ident[:, 0:64]), "transp")
                    _L(nc.tensor.transpose(pst[:, u, 64:128],
                                           emp[te][:, 2 * po:2 * po + 2, :],
                                        ident[:, 64:128]), "transp")
                F4 = fpool.tile([128, 4, BL], bf16, tag="F2")
                _L(nc.scalar.activation(F4[:], pst[:], AF.Exp,
                                     bias=bias_ap[:, 0:1]), "exp")
                return F4

            f4_next = make_f4(0)
            for p in range(S // 2):
                q, u = divmod(p, 4)
                if u == 0:
                    F4 = f4_next
                    if 4 * q + 4 < S // 2:
                        f4_next = make_f4(q + 1)
                F2 = F4[:, u, :]
                # apply any pending rescale to this tile's EVEN step rows
                for ch, zb in pending.pop(p, []):
                    c0, cw = offs[ch], cws[ch]
                    _L(nc.vector.tensor_mul(F2[0:NT, c0:c0 + cw],
                                         F2[0:NT, c0:c0 + cw],
                                         zb[:]), "applyz")
                for sub in (0, 1):
                    s = 2 * p + sub
                    ro = 64 * sub
                    for ch in range(NCH):
                        c0, cw = offs[ch], cws[ch]
                        psp = psP.tile([65, cw], f32, tag="psp")
                        _L(nc.tensor.matmul(psp[:], eaug[:], betas[ch][:],
                                         start=True, stop=True), f"mm{ch}")
                        if s % KRS == KRS - 8:  # capture normalizer
                            r = s // KRS
                            col = r * BL + c0
                            nc.scalar.activation(
                                zbuf[64:65, col:col + cw], psp[64:65, :],
                                AF.Copy)
                            rz = spool.tile([1, cw], f32, tag="rz")
                            nc.vector.reciprocal(rz[:], psp[64:65, :])
                            zb = spool.tile([NT, cw], f32, tag="zb")
                            nc.gpsimd.partition_broadcast(zb[:], rz[:],
                                                          channels=NT)
                            pending.setdefault(p + 2, []).append((ch, zb))
                        nb = bpool.tile([NT, cws[ch]], bf16, tag=f"beta{ch}")
                        _L(nc.vector.tensor_mul(nb[:], psp[0:NT, :],
                                             F2[ro:ro + NT, c0:c0 + cw]),
                           f"fwdmul{ch}")
                        betas[ch] = nb

            # ---- gold score (independent of forward) ----
            n_chunks = S // CH
            acc_all = cpool.tile([BL, n_chunks], f32, tag="acc_all")
            cnt_mms = []
            for c in range(n_chunks):
                width = CH + 1 if c < n_chunks - 1 else CH
                oh = ohpool.tile([BL, CH + 1, NT], bf16, tag="oh")
                tgr = ohpool.tile([BL, CH + 1, NT], i16, tag="tgr")
                tgv = tg16[:, c * CH:c * CH + width, None].broadcast_to(
                    [BL, width, NT])
                _L(nc.gpsimd.tensor_copy(tgr[:, :width, :], tgv), "tgbcast")
                _L(nc.vector.tensor_tensor(oh[:, :width, :],
                                           iota[:, :width, :],
                                           tgr[:, :width, :],
                                           OP.is_equal), "cmp")
                te = c // (EMT // CH)
                so = (c % (EMT // CH)) * CH
                junk = jpool.tile([BL, CH, NT], f32, tag="junk")
                _L(nc.gpsimd.tensor_tensor(junk[:],
                                        emp[te][:, so:so + CH, 0:NT],
                                        oh[:, :CH, :], OP.mult), "goldmul")
                nc.scalar.activation(junk[:], junk[:], AF.Copy,
                                     accum_out=acc_all[:, c:c + 1])
                npairs = width - 1
                for k in range(npairs // 2):
                    cnt_mms.append((oh, 2 * k, 2, 2 * k + 1, 2))
                if npairs % 2:
                    cnt_mms.append((oh, npairs - 1, 1, npairs, 1))
            gem = cpool.tile([BL, 1], f32, tag="gem")
            nc.vector.tensor_reduce(gem[:, 0:1], acc_all[:],
                                    mybir.AxisListType.XYZW, OP.add)

            cnt = psC.tile([96, 96], f32, tag="cnt")
            for idx, (oh, l0, lw, r0, rw) in enumerate(cnt_mms):
                nc.tensor.matmul(
                    cnt[0:48 * lw, 0:48 * rw],
                    oh[:, l0:l0 + lw, :],
                    oh[:, r0:r0 + rw, :],
                    start=(idx == 0),
                    stop=(idx == len(cnt_mms) - 1),
                    skip_group_check=True,
                )
            junk2 = jpool.tile([96, 96], f32, tag="junk2")
            gtr = cpool.tile([96, 1], f32, tag="gtr")
            nc.vector.tensor_mul(junk2[:], cnt[:], t2)
            nc.vector.tensor_reduce(gtr[:, 0:1], junk2[:],
                                    mybir.AxisListType.XYZW, OP.add)
            nc.sync.dma_start(out=gtr_d[:], in_=gtr[:])
            nc.sync.dma_start(out=gem_d[:], in_=gem[:])

            # ---- final: Sigma beta, batched Ln over all Z, reduce ----
            for ch in range(NCH):
                c0, cw = offs[ch], cws[ch]
                psf = psP.tile([65, cw], f32, tag="psp")
                nc.tensor.matmul(psf[:], eaug[:], betas[ch][:], start=True,
                                 stop=True)
                nc.vector.tensor_copy(
                    zbuf[64:65, NRS * BL + c0:NRS * BL + c0 + cw],
                    psf[64:65, :])
            lnb = cpool.tile([65, (NRS + 1) * BL], f32, tag="lnb")
            nc.scalar.activation(lnb[64:65, :], zbuf[64:65, :], AF.Ln)
            red = cpool.tile([65, BL], f32, tag="red")
            v = lnb[64:65, :].rearrange("p (r b) -> p b r", b=BL)
            nc.vector.tensor_reduce(red[64:65, :], v, mybir.AxisListType.X,
                                    OP.add)
            nc.sync.dma_start(out=logz_d[:], in_=red[64:65, :])

    nc.compile()
    return nc


def _numpy_reference(emissions, transitions, tags, mask):
    em = np.transpose(emissions, (1, 0, 2)).astype(np.float64)
    tg = tags.T.astype(np.int64)
    mk = mask.T.astype(np.float64)
    seq_len, batch, num_tags = em.shape
    emit = np.take_along_axis(em, tg[..., None], axis=2)[..., 0]
    trans = transitions[tg[:-1], tg[1:]].astype(np.float64)
    score = emit[0] + (emit[1:] * mk[1:]).sum(0) + (trans * mk[1:]).sum(0)
    alphas = np.full((batch, num_tags), -10000.0)
    alphas[:, 0] = 0.0
    T64 = transitions.astype(np.float64)
    for i in range(seq_len):
        x = alphas[:, :, None] + T64[None, :, :]
        m = x.max(axis=1)
        nxt = m + np.log(np.exp(x - m[:, None, :]).sum(axis=1)) + em[i]
        mi = mk[i][:, None]
        alphas = mi * nxt + (1.0 - mi) * alphas
    m = alphas.max(axis=1)
    logZ = m + np.log(np.exp(alphas - m[:, None]).sum(axis=1))
    return np.float32((logZ - score).mean())


def kernel(emissions, transitions, tags, mask):
    emissions = np.asarray(emissions, np.float32)
    transitions = np.asarray(transitions, np.float32)
    tags = np.asarray(tags, np.int32)
    mask_arr = np.asarray(mask)
    if not np.all(mask_arr == 1):
        return _numpy_reference(emissions, transitions, tags, mask_arr)

    from concourse.bass_utils import run_bass_kernel_spmd

    if "nc" not in _CACHE:
        _CACHE["nc"] = _build_nc()
    nc = _CACHE["nc"]

    E = np.exp(transitions.astype(np.float64)).astype(np.float32)
    consts = np.zeros((128, 418), np.float32)
    consts[:, 0:128] = np.eye(128, dtype=np.float32)
    consts[0:NT, 128:176] = E
    consts[0:NT, 192] = 1.0  # eaug ones column (col 64 of the eaug view)
    consts[0:48, 193:241] = transitions
    consts[48:96, 241:289] = transitions

    in_maps = []
    for i in range(NCORES):
        sl = slice(i * BL, (i + 1) * BL)
        in_maps.append({
            "em": np.ascontiguousarray(emissions[sl]),
            "tg": np.ascontiguousarray(tags[sl]),
            "consts": consts,
        })

    _CACHE["last_in_maps"] = in_maps
    res = run_bass_kernel_spmd(nc, in_maps, core_ids=list(range(NCORES)))
    logz = np.concatenate([r["logz"][0] for r in res.results])
    logz = logz.astype(np.float64) + S * EXP_BIAS
    gold = sum(float(r["goldem"].sum()) + float(r["goldtr"].sum())
               for r in res.results)
    loss = logz.mean() - gold / B
    return np.float32(loss)

